# revision 1
# baseline (speedup 1.0000x reference)
"""ChunkCrossAttention Trainium2 kernel.

Math (per reference):
  x = chunk_embeddings[0]                      # (S, L)
  k, v = split(x @ W_kv.T)                     # (S, D) each
  scores = einsum('jqd,sd->jqs', q, k) / sqrt(D), masked
  attn = softmax(scores, -1)
  out = (attn @ v) @ W_out.T + q  -> LayerNorm(gamma, beta)

Strategy (8 NeuronCores):
  - KV projection sharded over S: each core projects its own 512 keys
    (k^T, v^T in [d, s] layout straight out of the PE).
  - W_out folded into v: v' = v @ W_out.T, with two ones columns appended
    so the attention matmul also emits the softmax denominator (and the
    fp32r even-width ISA restriction is satisfied).
  - Attention partials: every core computes exp(q_all . k_loc) @ v'_loc
    over its local keys for ALL 8192 query rows (softmax without
    max-subtraction, mask folded into the Exp bias), then a ReduceScatter
    sums the partials across cores and hands each core its 1024-row
    q-shard. No K/V gather; the collective payload is tiny.
  - Two q-halves pipeline the second ReduceScatter behind the first
    half's epilogue.
  - Matmuls run as float32r (full-rate fp32 on the PE).
"""
import sys

sys.path.insert(0, "/opt/trn_rl_repo")

import numpy as np

import concourse.bacc as bacc
import concourse.mybir as mybir
import concourse.tile as tile
from concourse.bass_utils import run_bass_kernel_spmd

N_CORES = 8
J, Q, D = 64, 128, 256
S, L = 4096, 4096
S_LOC = S // N_CORES          # 512 keys per core
QR = (J // N_CORES) * Q       # 1024 query rows per core (output shard)
QALL = J * Q                  # 8192 query rows total
DP = D + 2                    # attention free dim: D outputs + denom + pad
LN_EPS = 1e-5
SCALE = 1.0 / np.sqrt(D)

F32 = mybir.dt.float32
F32R = mybir.dt.float32r
BF16 = mybir.dt.bfloat16
AF = mybir.ActivationFunctionType
ALU = mybir.AluOpType


def _r(ap):
    return ap.bitcast(F32R)


def build_program():
    nc = bacc.Bacc(None, num_devices=N_CORES)

    xT = nc.declare_dram_parameter("xT", [L, S_LOC], BF16, isOutput=False)
    wkvT = nc.declare_dram_parameter("wkvT", [L, 2 * D], BF16, isOutput=False)
    qT = nc.declare_dram_parameter("qT", [D, QALL], BF16, isOutput=False)
    qres = nc.declare_dram_parameter("qres", [QR, D], F32, isOutput=False)
    woutT = nc.declare_dram_parameter("woutT", [D, D], BF16, isOutput=False)
    maskb = nc.declare_dram_parameter("maskb", [128, S_LOC // 128], F32,
                                      isOutput=False)
    gamma = nc.declare_dram_parameter("gamma", [D], F32, isOutput=False)
    beta = nc.declare_dram_parameter("beta", [D], F32, isOutput=False)
    y = nc.declare_dram_parameter("y", [QR, D], F32, isOutput=True)

    # partial attention sums, one dram tensor per q-half:
    # slot c of rs_in{h} = partials for global q rows c*1024 + h*512 + [0,512)
    rs_in = [nc.dram_tensor(f"rs_in{h}", [N_CORES, QR // 2, DP], F32)
             for h in range(2)]
    rs_out = [nc.dram_tensor(f"rs_out{h}", [QR // 2, DP], F32)
              for h in range(2)]

    import concourse.bass as bass

    with tile.TileContext(nc) as tc:
        with tc.tile_pool(name="singles", bufs=1) as singles, \
             tc.tile_pool(name="xw", bufs=4) as xw, \
             tc.tile_pool(name="kv", bufs=1) as kvp, \
             tc.tile_pool(name="exp", bufs=4) as epool, \
             tc.tile_pool(name="part", bufs=3) as ppool, \
             tc.tile_pool(name="small", bufs=8) as small:

            # ---- phase 1: local K^T / V^T projection over the S shard ----
            ps1 = tc.tile_pool(name="ps_kv", bufs=1, space="PSUM")
            ps_kv = ps1.__enter__()
            acc = [ps_kv.tile([128, S_LOC], F32, tag=f"acc{h}", name=f"acc{h}")
                   for h in range(4)]
            n_big = L // 512
            for lb in range(n_big):
                xt = xw.tile([128, 4, S_LOC], BF16, tag="xt")
                nc.sync.dma_start(
                    out=xt,
                    in_=xT[lb * 512:(lb + 1) * 512, :].rearrange(
                        "(a p) s -> p a s", p=128))
                wt = xw.tile([128, 4, 2 * D], BF16, tag="wt")
                nc.sync.dma_start(
                    out=wt,
                    in_=wkvT[lb * 512:(lb + 1) * 512, :].rearrange(
                        "(a p) s -> p a s", p=128))
                for a in range(4):
                    first = lb == 0 and a == 0
                    last = lb == n_big - 1 and a == 3
                    for h in range(4):
                        nc.tensor.matmul(acc[h], wt[:, a, h * 128:(h + 1) * 128],
                                         xt[:, a, :], start=first, stop=last)
                if lb == 1:
                    xt_probe = xt

            # ---- constant / input loads ----
            # read-before-write probe: pins the 4MB qT load behind the lb=1
            # x-chunk so it does not steal SDMA bandwidth from the phase-1
            # stream during warmup
            qT_sb = singles.tile([128, 2, QALL], BF16)
            probe = small.tile([128, 1], BF16, tag="probe")
            nc.vector.tensor_copy(out=probe, in_=xt_probe[:, 0, 0:1])
            nc.vector.tensor_copy(out=probe, in_=qT_sb[:, 0, 0:1])
            nc.gpsimd.dma_start(out=qT_sb,
                              in_=qT.rearrange("(dc p) q -> p dc q", p=128))
            qres_sb = singles.tile([128, QR // 128, D], F32)
            nc.gpsimd.dma_start(out=qres_sb,
                              in_=qres.rearrange("(t p) d -> p t d", p=128))
            woutT_sb = singles.tile([128, 2, D], BF16)
            nc.gpsimd.dma_start(out=woutT_sb,
                              in_=woutT.rearrange("(dc p) d2 -> p dc d2", p=128))
            maskb_sb = singles.tile([128, S_LOC // 128], F32)
            nc.gpsimd.dma_start(out=maskb_sb, in_=maskb[:, :])
            g_ap = gamma[:]
            gamma_sb = singles.tile([128, D], F32)
            nc.gpsimd.dma_start(out=gamma_sb, in_=bass.AP(
                tensor=g_ap.tensor, offset=g_ap.offset,
                ap=[[0, 128], g_ap.ap[0]]))
            b_ap = beta[:]
            beta_sb = singles.tile([128, D], F32)
            nc.gpsimd.dma_start(out=beta_sb, in_=bass.AP(
                tensor=b_ap.tensor, offset=b_ap.offset,
                ap=[[0, 128], b_ap.ap[0]]))
            eps_sb = singles.tile([128, 1], F32)
            nc.vector.memset(eps_sb, LN_EPS)

            kT_loc = kvp.tile([128, 2, S_LOC], BF16)
            nc.scalar.copy(out=kT_loc[:, 0, :], in_=acc[0])
            nc.scalar.copy(out=kT_loc[:, 1, :], in_=acc[1])
            vT_loc = kvp.tile([128, 2, S_LOC], BF16)
            nc.scalar.copy(out=vT_loc[:, 0, :], in_=acc[2])
            nc.scalar.copy(out=vT_loc[:, 1, :], in_=acc[3])

            # ---- v' = v @ W_out.T, plus ones columns -> [s, DP] ----
            vp_sb = kvp.tile([128, 4, DP], BF16)
            nc.vector.memset(vp_sb, 1.0)
            for ss in range(4):
                pv = ps_kv.tile([128, D], F32, tag="pv", name="pv")
                for dc in range(2):
                    nc.tensor.matmul(
                        pv, vT_loc[:, dc, ss * 128:(ss + 1) * 128],
                        woutT_sb[:, dc, :], start=(dc == 0), stop=(dc == 1))
                nc.vector.tensor_copy(out=vp_sb[:, ss, 0:D], in_=pv)
            ps1.__exit__(None, None, None)

            # ---- phase 2: partial attention over local keys, all queries ----
            ps3 = tc.tile_pool(name="ps_at", bufs=1, space="PSUM")
            ps_at = ps3.__enter__()
            ps3b = tc.tile_pool(name="ps_sc", bufs=3, space="PSUM")
            ps_sc = ps3b.__enter__()

            n_st = S_LOC // 128                       # 4 local key tiles
            # process all even q-chunks (first RS half) then odd ones
            for qc in [0, 2, 4, 6, 8, 10, 12, 14, 1, 3, 5, 7, 9, 11, 13, 15]:
                h, c = qc % 2, qc // 2
                at = [ps_at.tile([128, DP], F32, tag=f"at{i}", name=f"at{i}")
                      for i in range(4)]
                for st in range(n_st):
                    sc = ps_sc.tile([128, 512], F32)
                    for dc in range(2):
                        nc.tensor.matmul(
                            sc, kT_loc[:, dc, st * 128:(st + 1) * 128],
                            qT_sb[:, dc, qc * 512:(qc + 1) * 512],
                            start=(dc == 0), stop=(dc == 1))
                    ex = epool.tile([128, 512], BF16)
                    nc.scalar.activation(out=ex, in_=sc, func=AF.Exp,
                                         bias=maskb_sb[:, st:st + 1], scale=SCALE)
                    for qt in range(4):
                        nc.tensor.matmul(
                            at[qt], ex[:, qt * 128:(qt + 1) * 128],
                            vp_sb[:, st, :],
                            start=(st == 0), stop=(st == n_st - 1))
                part = ppool.tile([128, 4, DP], F32, tag="part")
                for qt in range(4):
                    nc.vector.tensor_copy(out=part[:, qt, :], in_=at[qt])
                nc.sync.dma_start(
                    out=rs_in[h].rearrange("c (t p) f -> c p t f", p=128)[c],
                    in_=part)

            ps3b.__exit__(None, None, None)
            ps3.__exit__(None, None, None)

            # ---- phase 3: reduce-scatter partials, epilogue on own shard ----
            for h in range(2):
                nc.gpsimd.collective_compute(
                    "ReduceScatter", ALU.add,
                    replica_groups=[list(range(N_CORES))],
                    ins=[rs_in[h][:, :, :]], outs=[rs_out[h][:, :]])
            y_r = y.rearrange("(hh t p) d -> hh p t d", hh=2, p=128)
            for h in range(2):
                rs_sb = singles.tile([128, 4, DP], F32, name=f"rs_sb{h}")
                nc.gpsimd.dma_start(
                    out=rs_sb,
                    in_=rs_out[h].rearrange("(t p) f -> p t f", p=128))
                h_half = singles.tile([128, 4, D], F32, name=f"h_half{h}")
                for j in range(4):
                    t = 4 * h + j
                    hs = h_half[:, j, :]
                    rec = small.tile([128, 1], F32, tag="rec")
                    nc.vector.reciprocal(out=rec, in_=rs_sb[:, j, D:D + 1])
                    nc.vector.tensor_scalar_mul(out=hs, in0=rs_sb[:, j, 0:D],
                                                scalar1=rec)
                    nc.vector.tensor_add(out=hs, in0=hs, in1=qres_sb[:, t, :])
                    stats = small.tile([128, 6], F32, tag="stats")
                    nc.vector.bn_stats(out=stats, in_=hs)
                    mv = small.tile([128, 2], F32, tag="mv")
                    nc.vector.bn_aggr(out=mv, in_=stats)
                    rstd = small.tile([128, 1], F32, tag="rstd")
                    nc.scalar.activation(out=rstd, in_=mv[:, 1:2], func=AF.Sqrt,
                                         bias=eps_sb, scale=1.0)
                    nc.vector.reciprocal(out=rstd, in_=rstd)
                    nc.vector.tensor_scalar(out=hs, in0=hs,
                                            scalar1=mv[:, 0:1], scalar2=rstd,
                                            op0=ALU.subtract, op1=ALU.mult)
                    nc.vector.tensor_mul(out=hs, in0=hs, in1=gamma_sb)
                    nc.vector.tensor_add(out=hs, in0=hs, in1=beta_sb)
                nc.gpsimd.dma_start(out=y_r[h], in_=h_half)

    nc.finalize()
    return nc


_NC_CACHE = None


def _make_in_maps(inputs):
    jq = np.asarray(inputs["justice_queries"], dtype=np.float32)
    x = np.asarray(inputs["chunk_embeddings"], dtype=np.float32)[0]
    mask = np.asarray(inputs["chunk_mask"])
    wkv = np.asarray(inputs["W_kv"], dtype=np.float32)
    wout = np.asarray(inputs["W_out"], dtype=np.float32)
    gamma = np.asarray(inputs["ln_gamma"], dtype=np.float32)
    beta = np.asarray(inputs["ln_beta"], dtype=np.float32)

    import ml_dtypes
    bf16 = ml_dtypes.bfloat16
    xT = np.ascontiguousarray(x.T.astype(bf16))         # (L, S)
    wkvT = np.ascontiguousarray(wkv.T.astype(bf16))     # (L, 2D)
    flat = np.ascontiguousarray(jq.reshape(J * Q, D))   # (8192, D)
    qT = np.ascontiguousarray(flat.T.astype(bf16))      # (D, 8192)
    woutT = np.ascontiguousarray(wout.T.astype(bf16))   # (D, D)
    mb_full = np.where(mask != 0, 0.0, -1e30).astype(np.float32)

    in_maps = []
    for c in range(N_CORES):
        mb = mb_full[c * S_LOC:(c + 1) * S_LOC]
        in_maps.append({
            "xT": np.ascontiguousarray(xT[:, c * S_LOC:(c + 1) * S_LOC]),
            "wkvT": wkvT,
            "qT": qT,
            "qres": np.ascontiguousarray(flat[c * QR:(c + 1) * QR, :]),
            "woutT": woutT,
            "maskb": np.ascontiguousarray(mb.reshape(S_LOC // 128, 128).T),
            "gamma": gamma,
            "beta": beta,
        })
    return in_maps


def kernel(**inputs) -> np.ndarray:
    global _NC_CACHE
    in_maps = _make_in_maps(inputs)
    if _NC_CACHE is None:
        _NC_CACHE = build_program()
    res = run_bass_kernel_spmd(_NC_CACHE, in_maps, list(range(N_CORES)))
    out = np.concatenate([res.results[c]["y"] for c in range(N_CORES)], axis=0)
    return np.ascontiguousarray(out.reshape(J, Q, D).astype(np.float32))



# revision 2
# speedup vs baseline: 1.0490x; 1.0490x over previous
"""ChunkCrossAttention Trainium2 kernel.

Math (per reference):
  x = chunk_embeddings[0]                      # (S, L)
  k, v = split(x @ W_kv.T)                     # (S, D) each
  scores = einsum('jqd,sd->jqs', q, k) / sqrt(D), masked
  attn = softmax(scores, -1)
  out = (attn @ v) @ W_out.T + q  -> LayerNorm(gamma, beta)

Strategy (8 NeuronCores):
  - KV projection sharded over S: each core projects its own 512 keys
    (k^T, v^T in [d, s] layout straight out of the PE).
  - W_out folded into v: v' = v @ W_out.T, with two ones columns appended
    so the attention matmul also emits the softmax denominator.
  - Attention partials: every core computes exp(q_all . k_loc) @ v'_loc
    over its local keys for ALL 8192 query rows (softmax without
    max-subtraction, mask folded into the Exp bias).
  - Partials are exchanged in 4 pipelined WAVES of 2048 query rows each:
    bf16 AllToAll (copy-speed, ~2x a ReduceScatter) + a local f32
    vector reduction + LayerNorm epilogue, all overlapped with the next
    wave's attention compute.  Core c owns global rows
    g*2048 + c*256 + [0,256) of wave g (host reassembles).
  - Attention inner loop is software-pipelined (sc0,sc1,av0,sc2,av1,...)
    so the PE never waits on the scalar engine's Exp.
"""
import sys

sys.path.insert(0, "/opt/trn_rl_repo")

import numpy as np

import concourse.bacc as bacc
import concourse.mybir as mybir
import concourse.tile as tile
from concourse.bass_utils import run_bass_kernel_spmd

N_CORES = 8
J, Q, D = 64, 128, 256
S, L = 4096, 4096
S_LOC = S // N_CORES          # 512 keys per core
QALL = J * Q                  # 8192 query rows total
QR = QALL // N_CORES          # 1024 query rows per core (output shard)
NW = 4                        # waves
NCHUNK = 16                   # q-chunks of 512 rows
WROWS = QALL // NW            # 2048 global rows per wave
RW = WROWS // N_CORES         # 256 rows per core per wave
DP = D + 2                    # attention free dim: D outputs + denom + pad
LN_EPS = 1e-5
SCALE = 1.0 / np.sqrt(D)

F32 = mybir.dt.float32
BF16 = mybir.dt.bfloat16
AF = mybir.ActivationFunctionType
ALU = mybir.AluOpType


def build_program():
    nc = bacc.Bacc(None, num_devices=N_CORES)

    xT = nc.declare_dram_parameter("xT", [L, S_LOC], BF16, isOutput=False)
    wkvT = nc.declare_dram_parameter("wkvT", [L, 2 * D], BF16, isOutput=False)
    qT = nc.declare_dram_parameter("qT", [D, QALL], BF16, isOutput=False)
    qres = nc.declare_dram_parameter("qres", [QR, D], F32, isOutput=False)
    woutT = nc.declare_dram_parameter("woutT", [D, D], BF16, isOutput=False)
    maskb = nc.declare_dram_parameter("maskb", [128, S_LOC // 128], F32,
                                      isOutput=False)
    gamma = nc.declare_dram_parameter("gamma", [D], F32, isOutput=False)
    beta = nc.declare_dram_parameter("beta", [D], F32, isOutput=False)
    y = nc.declare_dram_parameter("y", [QR, D], F32, isOutput=True)

    # per-wave partial exchange buffers:
    # a2a_in[g] slot c = bf16 partials for global q rows g*2048+c*256+[0,256)
    # after AllToAll, a2a_out[g] slot j = core j's partials for OUR rows.
    a2a_in = [nc.dram_tensor(f"a2a_in{g}", [N_CORES, RW, DP], BF16)
              for g in range(NW)]
    a2a_out = [nc.dram_tensor(f"a2a_out{g}", [N_CORES, RW, DP], BF16)
               for g in range(NW)]

    import concourse.bass as bass

    with tile.TileContext(nc) as tc:
        with tc.tile_pool(name="singles", bufs=1) as singles, \
             tc.tile_pool(name="xw", bufs=4) as xw, \
             tc.tile_pool(name="kv", bufs=1) as kvp, \
             tc.tile_pool(name="exp", bufs=4) as epool, \
             tc.tile_pool(name="part", bufs=3) as ppool, \
             tc.tile_pool(name="wave", bufs=2) as wpool, \
             tc.tile_pool(name="small", bufs=8) as small:

            # ---- small constants first (gpsimd queue), then qT waves ----
            woutT_sb = singles.tile([128, 2, D], BF16)
            nc.gpsimd.dma_start(out=woutT_sb,
                                in_=woutT.rearrange("(dc p) d2 -> p dc d2",
                                                    p=128))
            maskb_sb = singles.tile([128, S_LOC // 128], F32)
            nc.gpsimd.dma_start(out=maskb_sb, in_=maskb[:, :])
            g_ap = gamma[:]
            gamma_sb = singles.tile([128, D], F32)
            nc.gpsimd.dma_start(out=gamma_sb, in_=bass.AP(
                tensor=g_ap.tensor, offset=g_ap.offset,
                ap=[[0, 128], g_ap.ap[0]]))
            b_ap = beta[:]
            beta_sb = singles.tile([128, D], F32)
            nc.gpsimd.dma_start(out=beta_sb, in_=bass.AP(
                tensor=b_ap.tensor, offset=b_ap.offset,
                ap=[[0, 128], b_ap.ap[0]]))
            eps_sb = singles.tile([128, 1], F32)
            nc.vector.memset(eps_sb, LN_EPS)

            # qT in 4 wave slices; qres (epilogue-only) after wave 1's slice
            qT_sb = singles.tile([128, 2, NCHUNK, 512], BF16)
            for g in range(NW):
                nc.gpsimd.dma_start(
                    out=qT_sb[:, :, 4 * g:4 * (g + 1), :],
                    in_=qT[:, g * WROWS:(g + 1) * WROWS].rearrange(
                        "(dc p) q -> p dc q", p=128))
                if g == 1:
                    qres_sb = singles.tile([128, QR // 128, D], F32)
                    nc.gpsimd.dma_start(
                        out=qres_sb,
                        in_=qres.rearrange("(t p) d -> p t d", p=128))

            # ---- phase 1: local K^T / V^T projection over the S shard ----
            # x/w stream on the sync queue; first chunk small so the PE
            # starts as early as possible.
            ps1 = tc.tile_pool(name="ps_kv", bufs=1, space="PSUM")
            ps_kv = ps1.__enter__()
            acc = [ps_kv.tile([128, S_LOC], F32, tag=f"acc{h}", name=f"acc{h}")
                   for h in range(4)]
            chunks = [(0, 128), (128, 384)] + [(512 * i, 512)
                                               for i in range(1, 8)]
            n_mm = sum(nr // 128 for _, nr in chunks) * 4
            mm_i = 0
            for (r0, nr) in chunks:
                na = nr // 128
                xt = xw.tile([128, na, S_LOC], BF16, tag=f"xt{na}")
                nc.sync.dma_start(
                    out=xt,
                    in_=xT[r0:r0 + nr, :].rearrange("(a p) s -> p a s", p=128))
                wt = xw.tile([128, na, 2 * D], BF16, tag=f"wt{na}")
                nc.sync.dma_start(
                    out=wt,
                    in_=wkvT[r0:r0 + nr, :].rearrange("(a p) s -> p a s",
                                                      p=128))
                for a in range(na):
                    for h in range(4):
                        nc.tensor.matmul(acc[h], wt[:, a, h * 128:(h + 1) * 128],
                                         xt[:, a, :], start=(mm_i == 0),
                                         stop=(mm_i == n_mm - 4 + h))
                    mm_i += 4

            kT_loc = kvp.tile([128, 2, S_LOC], BF16)
            nc.scalar.copy(out=kT_loc[:, 0, :], in_=acc[0])
            nc.scalar.copy(out=kT_loc[:, 1, :], in_=acc[1])
            vT_loc = kvp.tile([128, 2, S_LOC], BF16)
            nc.scalar.copy(out=vT_loc[:, 0, :], in_=acc[2])
            nc.scalar.copy(out=vT_loc[:, 1, :], in_=acc[3])

            # ---- v' = v @ W_out.T, plus ones columns -> [s, DP] ----
            vp_sb = kvp.tile([128, 4, DP], BF16)
            nc.vector.memset(vp_sb, 1.0)
            for ss in range(4):
                pv = ps_kv.tile([128, D], F32, tag="pv", name="pv")
                for dc in range(2):
                    nc.tensor.matmul(
                        pv, vT_loc[:, dc, ss * 128:(ss + 1) * 128],
                        woutT_sb[:, dc, :], start=(dc == 0), stop=(dc == 1))
                nc.vector.tensor_copy(out=vp_sb[:, ss, 0:D], in_=pv)
            ps1.__exit__(None, None, None)

            # ---- phase 2: partial attention, 16 chunks in 4 waves ----
            ps3 = tc.tile_pool(name="ps_at", bufs=1, space="PSUM")
            ps_at = ps3.__enter__()
            ps3b = tc.tile_pool(name="ps_sc", bufs=3, space="PSUM")
            ps_sc = ps3b.__enter__()

            n_st = S_LOC // 128                       # 4 local key tiles

            def scores(p, st):
                sc = ps_sc.tile([128, 512], F32, tag="sc")
                for dc in range(2):
                    nc.tensor.matmul(
                        sc, kT_loc[:, dc, st * 128:(st + 1) * 128],
                        qT_sb[:, dc, p, :], start=(dc == 0), stop=(dc == 1))
                ex = epool.tile([128, 512], BF16, tag="ex")
                nc.scalar.activation(out=ex, in_=sc, func=AF.Exp,
                                     bias=maskb_sb[:, st:st + 1], scale=SCALE)
                return ex

            for p in range(NCHUNK):
                g = p // 4
                at = [ps_at.tile([128, DP], F32, tag=f"at{i}", name=f"at{i}")
                      for i in range(4)]

                # software pipeline: sc0, sc1, av0, sc2, av1, sc3, av2, av3
                ex = [None] * n_st

                def av(st):
                    for qt in range(4):
                        nc.tensor.matmul(
                            at[qt], ex[st][:, qt * 128:(qt + 1) * 128],
                            vp_sb[:, st, :],
                            start=(st == 0), stop=(st == n_st - 1))

                ex[0] = scores(p, 0)
                ex[1] = scores(p, 1)
                av(0)
                ex[2] = scores(p, 2)
                av(1)
                ex[3] = scores(p, 3)
                av(2)
                av(3)

                part = ppool.tile([128, 4, DP], BF16, tag="part")
                for qt in range(4):
                    nc.vector.tensor_copy(out=part[:, qt, :], in_=at[qt])
                a2a_r = a2a_in[g].rearrange("s (t p) f -> s p t f", p=128)
                slot = 2 * (p % 4)
                nc.scalar.dma_start(out=a2a_r[slot], in_=part[:, 0:2, :])
                nc.scalar.dma_start(out=a2a_r[slot + 1], in_=part[:, 2:4, :])

                if p % 4 == 3:
                    nc.gpsimd.collective_compute(
                        "AllToAll", ALU.bypass,
                        replica_groups=[list(range(N_CORES))],
                        ins=[a2a_in[g][:, :, :]], outs=[a2a_out[g][:, :, :]])

            ps3b.__exit__(None, None, None)
            ps3.__exit__(None, None, None)

            # ---- phase 3: per-wave local reduce + epilogue ----
            y_r = y.rearrange("(w t p) d -> w p t d", w=NW, p=128)
            for g in range(NW):
                ao = wpool.tile([128, N_CORES * 2, DP], BF16, tag="ao")
                nc.scalar.dma_start(
                    out=ao,
                    in_=a2a_out[g].rearrange("s (t p) f -> p (s t) f", p=128))
                red = wpool.tile([128, 2, DP], F32, tag="red")
                # view [p, t, f, s]: slot index s has stride 2*DP in ao
                ao_v = ao.rearrange("p (s t) f -> p t f s", s=N_CORES)
                nc.vector.tensor_reduce(
                    out=red.rearrange("p t f -> p (t f)"), in_=ao_v,
                    axis=mybir.AxisListType.X, op=ALU.add)
                h_half = wpool.tile([128, 2, D], F32, tag="h")
                for t in range(2):
                    hs = h_half[:, t, :]
                    rec = small.tile([128, 1], F32, tag="rec")
                    nc.vector.reciprocal(out=rec, in_=red[:, t, D:D + 1])
                    nc.vector.tensor_scalar_mul(out=hs, in0=red[:, t, 0:D],
                                                scalar1=rec)
                    nc.vector.tensor_add(out=hs, in0=hs,
                                         in1=qres_sb[:, 2 * g + t, :])
                    stats = small.tile([128, 6], F32, tag="stats")
                    nc.vector.bn_stats(out=stats, in_=hs)
                    mv = small.tile([128, 2], F32, tag="mv")
                    nc.vector.bn_aggr(out=mv, in_=stats)
                    rstd = small.tile([128, 1], F32, tag="rstd")
                    nc.scalar.activation(out=rstd, in_=mv[:, 1:2], func=AF.Sqrt,
                                         bias=eps_sb, scale=1.0)
                    nc.vector.reciprocal(out=rstd, in_=rstd)
                    nc.vector.tensor_scalar(out=hs, in0=hs,
                                            scalar1=mv[:, 0:1], scalar2=rstd,
                                            op0=ALU.subtract, op1=ALU.mult)
                    nc.vector.tensor_mul(out=hs, in0=hs, in1=gamma_sb)
                    nc.vector.tensor_add(out=hs, in0=hs, in1=beta_sb)
                nc.scalar.dma_start(out=y_r[g], in_=h_half)

    nc.finalize()
    return nc


_NC_CACHE = None


def _make_in_maps(inputs):
    jq = np.asarray(inputs["justice_queries"], dtype=np.float32)
    x = np.asarray(inputs["chunk_embeddings"], dtype=np.float32)[0]
    mask = np.asarray(inputs["chunk_mask"])
    wkv = np.asarray(inputs["W_kv"], dtype=np.float32)
    wout = np.asarray(inputs["W_out"], dtype=np.float32)
    gamma = np.asarray(inputs["ln_gamma"], dtype=np.float32)
    beta = np.asarray(inputs["ln_beta"], dtype=np.float32)

    import ml_dtypes
    bf16 = ml_dtypes.bfloat16
    xT = np.ascontiguousarray(x.T.astype(bf16))         # (L, S)
    wkvT = np.ascontiguousarray(wkv.T.astype(bf16))     # (L, 2D)
    flat = np.ascontiguousarray(jq.reshape(J * Q, D))   # (8192, D)
    qT = np.ascontiguousarray(flat.T.astype(bf16))      # (D, 8192)
    woutT = np.ascontiguousarray(wout.T.astype(bf16))   # (D, D)
    mb_full = np.where(mask != 0, 0.0, -1e30).astype(np.float32)

    in_maps = []
    for c in range(N_CORES):
        mb = mb_full[c * S_LOC:(c + 1) * S_LOC]
        # core c owns global rows g*2048 + c*256 + [0,256) of wave g
        rows = np.concatenate([
            np.arange(g * WROWS + c * RW, g * WROWS + (c + 1) * RW)
            for g in range(NW)])
        in_maps.append({
            "xT": np.ascontiguousarray(xT[:, c * S_LOC:(c + 1) * S_LOC]),
            "wkvT": wkvT,
            "qT": qT,
            "qres": np.ascontiguousarray(flat[rows, :]),
            "woutT": woutT,
            "maskb": np.ascontiguousarray(mb.reshape(S_LOC // 128, 128).T),
            "gamma": gamma,
            "beta": beta,
        })
    return in_maps


def kernel(**inputs) -> np.ndarray:
    global _NC_CACHE
    in_maps = _make_in_maps(inputs)
    if _NC_CACHE is None:
        _NC_CACHE = build_program()
    res = run_bass_kernel_spmd(_NC_CACHE, in_maps, list(range(N_CORES)))
    out = np.empty((QALL, D), dtype=np.float32)
    for c in range(N_CORES):
        yc = res.results[c]["y"]
        for g in range(NW):
            out[g * WROWS + c * RW:g * WROWS + (c + 1) * RW] = \
                yc[g * RW:(g + 1) * RW]
    return np.ascontiguousarray(out.reshape(J, Q, D))


# revision 6
# speedup vs baseline: 1.1137x; 1.0617x over previous
"""ChunkCrossAttention Trainium2 kernel.

Math (per reference):
  x = chunk_embeddings[0]                      # (S, L)
  k, v = split(x @ W_kv.T)                     # (S, D) each
  scores = einsum('jqd,sd->jqs', q, k) / sqrt(D), masked
  attn = softmax(scores, -1)
  out = (attn @ v) @ W_out.T + q  -> LayerNorm(gamma, beta)

Strategy (8 NeuronCores):
  - KV projection sharded over S: each core projects its own 512 keys
    (k^T, v^T in [d, s] layout straight out of the PE).
  - W_out folded into v: v' = v @ W_out.T, with two ones columns appended
    so the attention matmul also emits the softmax denominator.
  - Attention partials: every core computes exp(q_all . k_loc) @ v'_loc
    over its local keys for ALL 8192 query rows (softmax without
    max-subtraction, mask folded into the Exp bias).
  - Partials are exchanged in 4 pipelined WAVES of 2048 query rows each:
    bf16 AllToAll (copy-speed, ~2x a ReduceScatter) + a local f32
    vector reduction + LayerNorm epilogue, all overlapped with the next
    wave's attention compute.  Core c owns global rows
    g*2048 + c*256 + [0,256) of wave g (host reassembles).
  - Attention inner loop is software-pipelined (sc0,sc1,av0,sc2,av1,...)
    so the PE never waits on the scalar engine's Exp.
"""
import sys

sys.path.insert(0, "/opt/trn_rl_repo")

import numpy as np

import concourse.bacc as bacc
import concourse.mybir as mybir
import concourse.tile as tile
from concourse.bass_utils import run_bass_kernel_spmd

N_CORES = 8
J, Q, D = 64, 128, 256
S, L = 4096, 4096
S_LOC = S // N_CORES          # 512 keys per core
QALL = J * Q                  # 8192 query rows total
QR = QALL // N_CORES          # 1024 query rows per core (output shard)
NW = 4                        # waves
NCHUNK = 16                   # q-chunks of 512 rows
WROWS = QALL // NW            # 2048 global rows per wave
RW = WROWS // N_CORES         # 256 rows per core per wave
DP = D + 2                    # attention free dim: D outputs + denom + pad
LN_EPS = 1e-5
SCALE = 1.0 / np.sqrt(D)

F32 = mybir.dt.float32
BF16 = mybir.dt.bfloat16
AF = mybir.ActivationFunctionType
ALU = mybir.AluOpType


def build_program():
    nc = bacc.Bacc(None, num_devices=N_CORES)

    xT = nc.declare_dram_parameter("xT", [L, S_LOC], BF16, isOutput=False)
    wkvT = nc.declare_dram_parameter("wkvT", [L, 2 * D], BF16, isOutput=False)
    qT = nc.declare_dram_parameter("qT", [D, QALL], BF16, isOutput=False)
    qres = nc.declare_dram_parameter("qres", [QR, D], F32, isOutput=False)
    woutT = nc.declare_dram_parameter("woutT", [D, D], BF16, isOutput=False)
    maskb = nc.declare_dram_parameter("maskb", [128, S_LOC // 128], F32,
                                      isOutput=False)
    gamma = nc.declare_dram_parameter("gamma", [D], F32, isOutput=False)
    beta = nc.declare_dram_parameter("beta", [D], F32, isOutput=False)
    y = nc.declare_dram_parameter("y", [QR, D], F32, isOutput=True)

    # per-wave partial exchange buffers:
    # a2a_in[g] slot c = bf16 partials for global q rows g*2048+c*256+[0,256)
    # after AllToAll, a2a_out[g] slot j = core j's partials for OUR rows.
    a2a_in = [nc.dram_tensor(f"a2a_in{g}", [N_CORES, RW, DP], BF16)
              for g in range(NW)]
    a2a_out = [nc.dram_tensor(f"a2a_out{g}", [N_CORES, RW, DP], BF16)
               for g in range(NW)]

    import concourse.bass as bass

    with tile.TileContext(nc) as tc:
        with tc.tile_pool(name="singles", bufs=1) as singles, \
             tc.tile_pool(name="xw", bufs=4) as xw, \
             tc.tile_pool(name="kv", bufs=1) as kvp, \
             tc.tile_pool(name="exp", bufs=4) as epool, \
             tc.tile_pool(name="part", bufs=3) as ppool, \
             tc.tile_pool(name="wave", bufs=2) as wpool, \
             tc.tile_pool(name="small", bufs=8) as small:

            # ---- small constants first (gpsimd queue), then qT waves ----
            woutT_sb = singles.tile([128, 2, D], BF16)
            nc.gpsimd.dma_start(out=woutT_sb,
                                in_=woutT.rearrange("(dc p) d2 -> p dc d2",
                                                    p=128))
            maskb_sb = singles.tile([128, S_LOC // 128], F32)
            nc.gpsimd.dma_start(out=maskb_sb, in_=maskb[:, :])
            g_ap = gamma[:]
            gamma_sb = singles.tile([128, D], F32)
            nc.gpsimd.dma_start(out=gamma_sb, in_=bass.AP(
                tensor=g_ap.tensor, offset=g_ap.offset,
                ap=[[0, 128], g_ap.ap[0]]))
            b_ap = beta[:]
            beta_sb = singles.tile([128, D], F32)
            nc.gpsimd.dma_start(out=beta_sb, in_=bass.AP(
                tensor=b_ap.tensor, offset=b_ap.offset,
                ap=[[0, 128], b_ap.ap[0]]))
            eps_sb = singles.tile([128, 1], F32)
            nc.vector.memset(eps_sb, LN_EPS)

            # qT wave 0 now; waves 1-3 and qres are probe-pinned into the
            # attention phase so they don't steal DMA bandwidth from the
            # x/w stream during the KV projection.
            qT_sb = singles.tile([128, 2, NCHUNK, 512], BF16)
            qres_sb = singles.tile([128, QR // 128, D], F32)

            def load_qT_wave(g):
                nc.gpsimd.dma_start(
                    out=qT_sb[:, :, 4 * g:4 * (g + 1), :],
                    in_=qT[:, g * WROWS:(g + 1) * WROWS].rearrange(
                        "(dc p) q -> p dc q", p=128))

            load_qT_wave(0)

            # ---- phase 1: local K^T / V^T projection over the S shard ----
            # x/w stream on the sync queue; first chunk small so the PE
            # starts as early as possible.
            ps1 = tc.tile_pool(name="ps_kv", bufs=1, space="PSUM")
            ps_kv = ps1.__enter__()
            acc = [ps_kv.tile([128, S_LOC], F32, tag=f"acc{h}", name=f"acc{h}")
                   for h in range(4)]
            chunks = [(0, 128), (128, 384)] + [(512 * i, 512)
                                               for i in range(1, 8)]
            n_mm = sum(nr // 128 for _, nr in chunks) * 4
            mm_i = 0
            for (r0, nr) in chunks:
                na = nr // 128
                xt = xw.tile([128, na, S_LOC], BF16, tag=f"xt{na}")
                nc.sync.dma_start(
                    out=xt,
                    in_=xT[r0:r0 + nr, :].rearrange("(a p) s -> p a s", p=128))
                wt = xw.tile([128, na, 2 * D], BF16, tag=f"wt{na}")
                nc.scalar.dma_start(
                    out=wt,
                    in_=wkvT[r0:r0 + nr, :].rearrange("(a p) s -> p a s",
                                                      p=128))
                for a in range(na):
                    for h in range(4):
                        nc.tensor.matmul(acc[h], wt[:, a, h * 128:(h + 1) * 128],
                                         xt[:, a, :], start=(mm_i == 0),
                                         stop=(mm_i == n_mm - 4 + h))
                    mm_i += 4

            kT_loc = kvp.tile([128, 2, S_LOC], BF16)
            nc.scalar.copy(out=kT_loc[:, 0, :], in_=acc[0])
            nc.scalar.copy(out=kT_loc[:, 1, :], in_=acc[1])
            vT_loc = kvp.tile([128, 2, S_LOC], BF16)
            nc.scalar.copy(out=vT_loc[:, 0, :], in_=acc[2])
            nc.scalar.copy(out=vT_loc[:, 1, :], in_=acc[3])

            # ---- v' = v @ W_out.T, plus ones columns -> [s, DP] ----
            vp_sb = kvp.tile([128, 4, DP], BF16)
            nc.vector.memset(vp_sb, 1.0)
            for ss in range(4):
                pv = ps_kv.tile([128, D], F32, tag="pv", name="pv")
                for dc in range(2):
                    nc.tensor.matmul(
                        pv, vT_loc[:, dc, ss * 128:(ss + 1) * 128],
                        woutT_sb[:, dc, :], start=(dc == 0), stop=(dc == 1))
                nc.vector.tensor_copy(out=vp_sb[:, ss, 0:D], in_=pv)
            ps1.__exit__(None, None, None)

            # ---- phase 2: partial attention, 16 chunks in 4 waves ----
            ps3 = tc.tile_pool(name="ps_at", bufs=1, space="PSUM")
            ps_at = ps3.__enter__()
            ps3b = tc.tile_pool(name="ps_sc", bufs=3, space="PSUM")
            ps_sc = ps3b.__enter__()

            n_st = S_LOC // 128                       # 4 local key tiles

            def scores(p, st):
                sc = ps_sc.tile([128, 512], F32, tag="sc")
                for dc in range(2):
                    nc.tensor.matmul(
                        sc, kT_loc[:, dc, st * 128:(st + 1) * 128],
                        qT_sb[:, dc, p, :], start=(dc == 0), stop=(dc == 1))
                ex = epool.tile([128, 512], BF16, tag="ex")
                nc.scalar.activation(out=ex, in_=sc, func=AF.Exp,
                                     bias=maskb_sb[:, st:st + 1], scale=SCALE)
                return ex

            for p in range(NCHUNK):
                g = p // 4
                at = [ps_at.tile([128, DP], F32, tag=f"at{i}", name=f"at{i}")
                      for i in range(4)]

                # software pipeline: sc0, sc1, av0, sc2, av1, sc3, av2, av3
                ex = [None] * n_st

                def av(st):
                    for qt in range(4):
                        nc.tensor.matmul(
                            at[qt], ex[st][:, qt * 128:(qt + 1) * 128],
                            vp_sb[:, st, :],
                            start=(st == 0), stop=(st == n_st - 1))

                ex[0] = scores(p, 0)
                ex[1] = scores(p, 1)
                av(0)
                ex[2] = scores(p, 2)
                av(1)
                ex[3] = scores(p, 3)
                av(2)
                av(3)

                part = ppool.tile([128, 4, DP], BF16, tag="part")
                for qt in range(4):
                    nc.vector.tensor_copy(out=part[:, qt, :], in_=at[qt])
                a2a_r = a2a_in[g].rearrange("s (t p) f -> s p t f", p=128)
                slot = 2 * (p % 4)
                nc.scalar.dma_start(out=a2a_r[slot], in_=part[:, 0:2, :])
                nc.scalar.dma_start(out=a2a_r[slot + 1], in_=part[:, 2:4, :])

                # probe-pinned late loads: a read of the DMA's target region
                # creates a WAR dependency, so the transfer can't start until
                # this point in the attention pipeline (keeps the x/w stream
                # alone on HBM during the KV projection).
                if p in (0, 2, 4, 8):
                    probe = small.tile([128, 1], BF16, tag="probe")
                    if p == 2:
                        qprobe = small.tile([128, 1], F32, tag="qprobe")
                        nc.vector.tensor_copy(out=qprobe,
                                              in_=qres_sb[:, 0, 0:1])
                        nc.gpsimd.dma_start(
                            out=qres_sb,
                            in_=qres.rearrange("(t p) d -> p t d", p=128))
                    else:
                        gload = {0: 1, 4: 2, 8: 3}[p]
                        nc.vector.tensor_copy(
                            out=probe, in_=qT_sb[:, 0, 4 * gload, 0:1])
                        load_qT_wave(gload)

                if p % 4 == 3:
                    nc.gpsimd.collective_compute(
                        "AllToAll", ALU.bypass,
                        replica_groups=[list(range(N_CORES))],
                        ins=[a2a_in[g][:, :, :]], outs=[a2a_out[g][:, :, :]])

            ps3b.__exit__(None, None, None)
            ps3.__exit__(None, None, None)

            # ---- phase 3: per-wave local reduce + epilogue ----
            y_r = y.rearrange("(w t p) d -> w p t d", w=NW, p=128)
            for g in range(NW):
                ao = wpool.tile([128, N_CORES * 2, DP], BF16, tag="ao")
                nc.scalar.dma_start(
                    out=ao,
                    in_=a2a_out[g].rearrange("s (t p) f -> p (s t) f", p=128))
                red = wpool.tile([128, 2, DP], F32, tag="red")
                # view [p, t, f, s]: slot index s has stride 2*DP in ao
                ao_v = ao.rearrange("p (s t) f -> p t f s", s=N_CORES)
                nc.vector.tensor_reduce(
                    out=red.rearrange("p t f -> p (t f)"), in_=ao_v,
                    axis=mybir.AxisListType.X, op=ALU.add)
                h_half = wpool.tile([128, 2, D], F32, tag="h")
                for t in range(2):
                    hs = h_half[:, t, :]
                    rec = small.tile([128, 1], F32, tag="rec")
                    nc.vector.reciprocal(out=rec, in_=red[:, t, D:D + 1])
                    nc.vector.tensor_scalar_mul(out=hs, in0=red[:, t, 0:D],
                                                scalar1=rec)
                    nc.vector.tensor_add(out=hs, in0=hs,
                                         in1=qres_sb[:, 2 * g + t, :])
                    stats = small.tile([128, 6], F32, tag="stats")
                    nc.vector.bn_stats(out=stats, in_=hs)
                    mv = small.tile([128, 2], F32, tag="mv")
                    nc.vector.bn_aggr(out=mv, in_=stats)
                    rstd = small.tile([128, 1], F32, tag="rstd")
                    nc.scalar.activation(out=rstd, in_=mv[:, 1:2], func=AF.Sqrt,
                                         bias=eps_sb, scale=1.0)
                    nc.vector.reciprocal(out=rstd, in_=rstd)
                    nc.vector.tensor_scalar(out=hs, in0=hs,
                                            scalar1=mv[:, 0:1], scalar2=rstd,
                                            op0=ALU.subtract, op1=ALU.mult)
                    nc.vector.tensor_mul(out=hs, in0=hs, in1=gamma_sb)
                    nc.vector.tensor_add(out=hs, in0=hs, in1=beta_sb)
                nc.scalar.dma_start(out=y_r[g], in_=h_half)

    nc.finalize()
    return nc


_NC_CACHE = None


def _make_in_maps(inputs):
    jq = np.asarray(inputs["justice_queries"], dtype=np.float32)
    x = np.asarray(inputs["chunk_embeddings"], dtype=np.float32)[0]
    mask = np.asarray(inputs["chunk_mask"])
    wkv = np.asarray(inputs["W_kv"], dtype=np.float32)
    wout = np.asarray(inputs["W_out"], dtype=np.float32)
    gamma = np.asarray(inputs["ln_gamma"], dtype=np.float32)
    beta = np.asarray(inputs["ln_beta"], dtype=np.float32)

    import ml_dtypes
    bf16 = ml_dtypes.bfloat16
    xT = np.ascontiguousarray(x.T.astype(bf16))         # (L, S)
    wkvT = np.ascontiguousarray(wkv.T.astype(bf16))     # (L, 2D)
    flat = np.ascontiguousarray(jq.reshape(J * Q, D))   # (8192, D)
    qT = np.ascontiguousarray(flat.T.astype(bf16))      # (D, 8192)
    woutT = np.ascontiguousarray(wout.T.astype(bf16))   # (D, D)
    mb_full = np.where(mask != 0, 0.0, -1e30).astype(np.float32)

    in_maps = []
    for c in range(N_CORES):
        mb = mb_full[c * S_LOC:(c + 1) * S_LOC]
        # core c owns global rows g*2048 + c*256 + [0,256) of wave g
        rows = np.concatenate([
            np.arange(g * WROWS + c * RW, g * WROWS + (c + 1) * RW)
            for g in range(NW)])
        in_maps.append({
            "xT": np.ascontiguousarray(xT[:, c * S_LOC:(c + 1) * S_LOC]),
            "wkvT": wkvT,
            "qT": qT,
            "qres": np.ascontiguousarray(flat[rows, :]),
            "woutT": woutT,
            "maskb": np.ascontiguousarray(mb.reshape(S_LOC // 128, 128).T),
            "gamma": gamma,
            "beta": beta,
        })
    return in_maps


def kernel(**inputs) -> np.ndarray:
    global _NC_CACHE
    in_maps = _make_in_maps(inputs)
    if _NC_CACHE is None:
        _NC_CACHE = build_program()
    res = run_bass_kernel_spmd(_NC_CACHE, in_maps, list(range(N_CORES)))
    out = np.empty((QALL, D), dtype=np.float32)
    for c in range(N_CORES):
        yc = res.results[c]["y"]
        for g in range(NW):
            out[g * WROWS + c * RW:g * WROWS + (c + 1) * RW] = \
                yc[g * RW:(g + 1) * RW]
    return np.ascontiguousarray(out.reshape(J, Q, D))
